# revision 15
# baseline (speedup 1.0000x reference)
"""Trainium2 Bass kernel: DeepIterativeNetwork (GNN message passing).

Strategy (8 NeuronCores, SPMD):
  - Data-parallel over the node axis N: each core owns MP=12800 node slots
    (12500 real + 300 pad).
  - All per-node activations live feature-major in SBUF ([128=E partitions,
    nodes on the free axis]); weights are stationary lhsT tiles.
  - Per message-passing iteration, each core writes its V-shard (bf16,
    transpose-tiled layout) to DRAM and AllGathers it piecewise so every core
    sees the full V for the neighbor gather.  The neighbor gather is an
    indirect DMA of 512B rows; the 4-neighbor sum + transpose back to
    feature-major is fused into PE transpose-matmuls accumulating in PSUM.
  - Host does index-only preprocessing (neighbor ids, layout remap) and
    parameter staging (transpose / cast / stacking); all floating-point math
    on the N-sized data runs on device.
"""

import math
import os
import sys

for _p in ("/opt/trn_rl_repo", "/root/.axon_site/_ro/trn_rl_repo"):
    if os.path.isdir(_p) and _p not in sys.path:
        sys.path.insert(0, _p)

import numpy as np
import ml_dtypes

import concourse.bass as bass
import concourse.bacc as bacc
import concourse.tile as tile
from concourse import mybir
from concourse.bass_utils import run_bass_kernel_spmd

BF16 = ml_dtypes.bfloat16
F32 = np.float32

AF = mybir.ActivationFunctionType
OP = mybir.AluOpType

# ---------------------------------------------------------------- config

class Cfg:
    def __init__(self, ncores=8, m_real=12500, mp=12800, npiece=5, nb=200000,
                 t_iters=3):
        self.ncores = ncores
        self.m_real = m_real          # real nodes per core
        self.mp = mp                  # padded nodes per core (mult of 512)
        self.e = 128
        self.d = 4
        self.t_iters = t_iters
        self.f = 512                  # chunk size (nodes per chunk)
        self.nchunk = mp // self.f
        self.npiece = npiece          # all-gather pieces per iteration
        assert self.nchunk % npiece == 0
        self.cpp = self.nchunk // npiece   # chunks per piece
        self.gpc = self.f // 128           # groups (of 128 nodes) per chunk
        self.nb = nb

    @property
    def n_total(self):
        return self.ncores * self.m_real


FULL = Cfg()

# ---------------------------------------------------------------- host prep

def tiled_row(cfg, l):
    """Flat 128-value row index of local node l inside a core's V-shard DRAM
    buffer ([nchunk,128,gpc,128] transpose-tiled layout)."""
    c = l // cfg.f
    r = l % cfg.f
    gg = r // 128
    p = r % 128
    return c * cfg.f + p * cfg.gpc + gg


def global_row(cfg, n):
    """Row index of node n inside the piecewise-allgathered Vfull buffer."""
    rho = n // cfg.m_real
    l = n % cfg.m_real
    c = l // cfg.f
    q = c // cfg.cpp
    rows_per_piece_rank = cfg.cpp * cfg.f
    in_piece = tiled_row(cfg, l) - q * rows_per_piece_rank
    return (q * cfg.ncores + rho) * rows_per_piece_rank + in_piece


def prep_host(cfg, inputs):
    """Index preprocessing + parameter staging + sharding. Returns
    (scalars_dict, replicated_map, per_core_maps)."""
    N = cfg.n_total
    dev_ids = np.arange(N, dtype=np.int64)
    dbi = np.asarray(inputs["dev_breaker_ids"])
    bp = np.asarray(inputs["breaker_pairs"])
    ends = bp[dbi]                                    # (N, D, 2)
    nbr = np.where(ends[..., 0] == dev_ids[:, None], ends[..., 1],
                   ends[..., 0]).astype(np.int64)     # (N, D)

    # remap neighbor node ids -> rows of the allgathered Vfull buffer
    grow = np.empty(N, dtype=np.int64)
    for n0 in range(0, N, 1 << 20):
        n1 = min(N, n0 + (1 << 20))
        nn = np.arange(n0, n1)
        rho = nn // cfg.m_real
        l = nn % cfg.m_real
        c = l // cfg.f
        q = c // cfg.cpp
        rr = l % cfg.f
        tiled = c * cfg.f + (rr % 128) * cfg.gpc + (rr // 128)
        rpp = cfg.cpp * cfg.f
        grow[n0:n1] = (q * cfg.ncores + rho) * rpp + (tiled - q * rpp)
    nbr_rows = grow[nbr]                              # (N, D) int64

    cbs = np.asarray(inputs["breaker_state"])[dbi].astype(F32)   # (N, D)
    ps = np.asarray(inputs["protector_state"]).astype(F32)       # (N, 3)

    # parameters ------------------------------------------------------
    W0 = np.asarray(inputs["W0"], F32); b0 = np.asarray(inputs["b0"], F32)
    w1 = np.asarray(inputs["w1"], F32); b1 = np.asarray(inputs["b1"], F32)
    w2 = np.asarray(inputs["w2"], F32); b2 = np.asarray(inputs["b2"], F32)
    W3 = np.asarray(inputs["W3"], F32); b3 = np.asarray(inputs["b3"], F32)
    w4 = np.asarray(inputs["w4"], F32); b4 = np.asarray(inputs["b4"], F32)
    W5 = np.asarray(inputs["W5"], F32); b5 = np.asarray(inputs["b5"], F32)
    wc = np.asarray(inputs["wc"], F32); bc = float(np.asarray(inputs["bc"]))
    Wu = np.asarray(inputs["Wu"], F32); bu = np.asarray(inputs["bu"], F32)
    Wg = np.asarray(inputs["Wg"], F32); bg = np.asarray(inputs["bg"], F32)
    v0 = np.asarray(inputs["v0"], F32)

    E = cfg.e
    rep = {
        "W0T": np.ascontiguousarray(W0.T),
        "W3T": np.ascontiguousarray(W3.T),
        "WgT": np.ascontiguousarray(Wg.T),
        "W5Tb": np.ascontiguousarray(W5.T).astype(BF16),
        "WuTb": np.ascontiguousarray(Wu.T).reshape(2, E, 3 * E).astype(BF16),
        # [3,2,128]: (w4,b4), (w1,b1), (w2,b2)
        "augw": np.stack([np.stack([w4, b4]), np.stack([w1, b1]),
                          np.stack([w2, b2])]).astype(BF16),
        "burow": bu.reshape(3, 1, E).astype(F32),                 # [3,1,128]
        # bcol cols: b0, b3, b5, buN
        "bcol": np.stack([b0, b3, b5, bu[2 * E:]], axis=1).astype(F32),
        "v0col": v0.reshape(E, 1).astype(F32),
    }
    scalars = {"wc0": float(wc[0]), "wc1": float(wc[1]), "wc2": float(wc[2]),
               "bc": bc, "bg": bg, "v0": v0}

    per_core = []
    ones = np.ones(cfg.mp, F32)
    for cidx in range(cfg.ncores):
        s = cidx * cfg.m_real
        epad = cfg.mp - cfg.m_real
        cb = np.pad(cbs[s:s + cfg.m_real], ((0, epad), (0, 0))).T  # [4, MP]
        pst = np.pad(ps[s:s + cfg.m_real], ((0, epad), (0, 0))).T  # [3, MP]
        # packed outer-product rhs: per chunk 7 segments of F cols,
        # row 0 = data (cbs0..3, ps0..2), row 1 = ones
        seg = np.concatenate([cb, pst], axis=0)      # [7, MP]
        seg = seg.reshape(7, cfg.nchunk, cfg.f)
        preaug = np.empty((2, cfg.nchunk, 7, cfg.f), F32)
        preaug[0] = seg.transpose(1, 0, 2)
        preaug[1] = 1.0
        preaug = preaug.reshape(2, cfg.nchunk * 7 * cfg.f).astype(BF16)
        # gather indices [128, nchunk*gpc*4]
        nr = np.zeros((cfg.mp, cfg.d), np.int64)
        nr[:cfg.m_real] = nbr_rows[s:s + cfg.m_real]
        gidx = np.zeros((128, cfg.nchunk * cfg.gpc * cfg.d), np.int32)
        for c in range(cfg.nchunk):
            for gg in range(cfg.gpc):
                l0 = c * cfg.f + gg * 128
                col = (c * cfg.gpc + gg) * cfg.d
                gidx[:, col:col + cfg.d] = nr[l0:l0 + 128]
        per_core.append({
            "preaug": preaug, "gidx": gidx,
        })
    return scalars, rep, per_core


# ---------------------------------------------------------------- program

def build_program(cfg, sc):
    """Build the SPMD Bass program. sc: dict with wc0..2, bc (python floats
    baked as immediates)."""
    E, F, D = cfg.e, cfg.f, cfg.d
    nc = bacc.Bacc(None, num_devices=cfg.ncores)
    f32, bf16, i32 = mybir.dt.float32, mybir.dt.bfloat16, mybir.dt.int32

    # ---- I/O ----
    dp = nc.declare_dram_parameter
    W0T = dp("W0T", [E, E], f32, isOutput=False)
    W3T = dp("W3T", [E, E], f32, isOutput=False)
    WgT = dp("WgT", [E, E], f32, isOutput=False)
    W5Tb = dp("W5Tb", [E, E], bf16, isOutput=False)
    WuTb = dp("WuTb", [2, E, 3 * E], bf16, isOutput=False)
    augw = dp("augw", [3, 2, E], bf16, isOutput=False)
    burow = dp("burow", [3, 1, E], f32, isOutput=False)
    bcol = dp("bcol", [E, 4], f32, isOutput=False)
    v0col = dp("v0col", [E, 1], f32, isOutput=False)
    preaug_d = dp("preaug", [2, cfg.nchunk * 7 * cfg.f], bf16, isOutput=False)
    gidx_d = dp("gidx", [128, cfg.nchunk * cfg.gpc * D], i32, isOutput=False)
    Vout = dp("Vout", [cfg.nchunk, 128, F], f32, isOutput=True)
    gpart = dp("gpart", [E, 1], f32, isOutput=True)

    # ---- internal DRAM ----
    n_ag = cfg.t_iters - 1
    rows_full = cfg.ncores * cfg.mp
    Vsh = [nc.dram_tensor(f"Vsh{t}", [cfg.nchunk, 128, F], bf16)
           for t in range(n_ag)]
    Vfull = [nc.dram_tensor(f"Vfull{t}", [rows_full, E], bf16,
                            addr_space="Shared") for t in range(n_ag)]
    rg = [list(range(cfg.ncores))]

    with tile.TileContext(nc) as tc:
        with tc.tile_pool(name="resid", bufs=1) as resid:
            # residents ------------------------------------------------
            VT = resid.tile([128, cfg.mp], f32)
            PB = resid.tile([128, cfg.mp], f32)
            gidx_sb = resid.tile([128, cfg.nchunk * cfg.gpc * D], i32)
            nc.sync.dma_start(out=gidx_sb[:], in_=gidx_d[:])
            w0t = resid.tile([E, E], f32)
            nc.sync.dma_start(out=w0t[:], in_=W0T[:])
            w3t = resid.tile([E, E], f32)
            nc.sync.dma_start(out=w3t[:], in_=W3T[:])
            wgt = resid.tile([E, E], f32)
            nc.sync.dma_start(out=wgt[:], in_=WgT[:])
            w5tb = resid.tile([E, E], bf16)
            nc.sync.dma_start(out=w5tb[:], in_=W5Tb[:])
            wutb0 = resid.tile([E, 3 * E], bf16)
            nc.sync.dma_start(out=wutb0[:], in_=WuTb[0])
            wutb1 = resid.tile([E, 3 * E], bf16)
            nc.sync.dma_start(out=wutb1[:], in_=WuTb[1])
            augw_sb = [resid.tile([2, E], bf16, tag=f"augw{i}",
                                  name=f"augw{i}") for i in range(3)]
            for i in range(3):
                nc.sync.dma_start(out=augw_sb[i][:], in_=augw[i])
            burow_sb = [resid.tile([1, E], f32, tag=f"burow{i}",
                                   name=f"burow{i}") for i in range(3)]
            for i in range(3):
                nc.sync.dma_start(out=burow_sb[i][:], in_=burow[i])
            bcol_sb = resid.tile([E, 4], f32)
            nc.sync.dma_start(out=bcol_sb[:], in_=bcol[:])
            v0_sb = resid.tile([E, 1], f32)
            nc.sync.dma_start(out=v0_sb[:], in_=v0col[:])
            identb = resid.tile([E, E], bf16)
            identf = resid.tile([E, E], f32)
            ones_row = resid.tile([1, F], f32)
            nc.vector.memset(ones_row[:], 1.0)
            st0 = resid.tile([128, F], bf16)
            nc.vector.tensor_scalar(st0[:],
                                    v0_sb[:].to_broadcast([128, F]),
                                    float(D), None, OP.mult)
            from concourse.masks import make_identity
            make_identity(nc, identf[:])
            nc.vector.tensor_copy(identb[:], identf[:])

            # init resident V
            nc.vector.tensor_copy(VT[:, :], v0_sb[:].to_broadcast([128, cfg.mp]))

            # ---- preamble: compute PB = wc0*pe + wc1*be + bc ----
            with tc.tile_pool(name="pre_ps", bufs=2, space="PSUM") as pps, \
                 tc.tile_pool(name="pre_po", bufs=2, space="PSUM") as ppo, \
                 tc.tile_pool(name="pre_sb", bufs=3) as psb:
                for c in range(cfg.nchunk):
                    cs = slice(c * F, (c + 1) * F)
                    aug = psb.tile([2, 7 * F], bf16, tag="aug")
                    a0 = c * 7 * F
                    nc.sync.dma_start(out=aug[:], in_=preaug_d[:, a0:a0 + 7 * F])
                    H = psb.tile([128, 8 * F], bf16, tag="H")
                    # 4 psum tiles of 2 segments each
                    for half in range(4):
                        sp = pps.tile([128, 2 * F], f32, tag="seg")
                        for k in range(2):
                            seg = half * 2 + k
                            outap = sp[:, k * F:(k + 1) * F]
                            if seg < 4:      # be_d , d = seg
                                nc.tensor.matmul(
                                    outap, lhsT=augw_sb[0][:],
                                    rhs=aug[:, seg * F:(seg + 1) * F],
                                    start=True, stop=True)
                            elif seg < 7:    # pe_j , j = seg-4
                                nc.tensor.matmul(
                                    outap, lhsT=augw_sb[1][:],
                                    rhs=aug[:, seg * F:(seg + 1) * F],
                                    start=True, stop=True)
                            else:            # tb (cbs_sum)
                                nc.tensor.matmul(
                                    outap, lhsT=augw_sb[2][:],
                                    rhs=aug[:, 0:F], start=True, stop=False)
                                for dd in range(1, 4):
                                    nc.tensor.matmul(
                                        outap, lhsT=augw_sb[2][0:1, :],
                                        rhs=aug[0:1, dd * F:(dd + 1) * F],
                                        start=False, stop=(dd == 3))
                        nc.scalar.activation(H[:, half * 2 * F:(half + 1) * 2 * F],
                                             sp[:], AF.Tanh)
                    A = psb.tile([128, F], f32, tag="A")
                    nc.vector.tensor_reduce(
                        A[:], H[:, 0:4 * F].rearrange("p (d r) -> p r d", d=4),
                        axis=mybir.AxisListType.X, op=OP.add)
                    Pp = psb.tile([128, F], f32, tag="Pp")
                    nc.vector.tensor_reduce(
                        Pp[:], H[:, 4 * F:8 * F].rearrange("p (d r) -> p r d", d=4),
                        axis=mybir.AxisListType.X, op=OP.add)
                    P = psb.tile([128, F], f32, tag="P")
                    nc.vector.scalar_tensor_tensor(
                        P[:], in0=H[:, 7 * F:8 * F], scalar=2.0, in1=Pp[:],
                        op0=OP.mult, op1=OP.add)
                    bp = ppo.tile([128, 2 * F], f32, tag="bp")
                    nc.tensor.matmul(bp[:, :F], lhsT=w3t[:], rhs=A[:],
                                     start=True, stop=True)
                    nc.tensor.matmul(bp[:, F:], lhsT=w0t[:], rhs=P[:],
                                     start=True, stop=True)
                    bepe = psb.tile([128, 2 * F], f32, tag="bepe")
                    nc.scalar.activation(bepe[:, :F], bp[:, :F], AF.Tanh,
                                         bias=bcol_sb[:, 1:2])
                    nc.scalar.activation(bepe[:, F:], bp[:, F:], AF.Tanh,
                                         bias=bcol_sb[:, 0:1])
                    t1 = psb.tile([128, F], f32, tag="A", name="t1")
                    nc.vector.tensor_scalar(t1[:], bepe[:, F:],
                                            sc["wc0"], sc["bc"],
                                            OP.mult, OP.add)
                    nc.vector.scalar_tensor_tensor(
                        PB[:, cs], in0=bepe[:, :F], scalar=sc["wc1"],
                        in1=t1[:], op0=OP.mult, op1=OP.add)

            # ---- iterations ----
            with tc.tile_pool(name="ps_st", bufs=2, space="PSUM") as pst_pool, \
                 tc.tile_pool(name="ps_ne", bufs=1, space="PSUM") as pne, \
                 tc.tile_pool(name="ps_ku", bufs=1, space="PSUM") as pku, \
                 tc.tile_pool(name="ps_nw", bufs=1, space="PSUM") as pnw, \
                 tc.tile_pool(name="ps_vo", bufs=2, space="PSUM") as pvo, \
                 tc.tile_pool(name="it_sb", bufs=2) as isb, \
                 tc.tile_pool(name="g_sb", bufs=8) as gsb:
                for t in range(cfg.t_iters):
                    vfull_t = Vfull[t - 1] if t > 0 else None
                    for c in range(cfg.nchunk):
                        cs = slice(c * F, (c + 1) * F)
                        if t == 0:
                            stbf = st0
                        else:
                            gts = []
                            for gg in range(cfg.gpc):
                                gt = gsb.tile([128, D * 128], bf16, tag="g")
                                col = (c * cfg.gpc + gg) * D
                                nc.gpsimd.indirect_dma_start(
                                    out=gt[:], out_offset=None,
                                    in_=vfull_t[:, :],
                                    in_offset=bass.IndirectOffsetOnAxis(
                                        ap=gidx_sb[:, col:col + D], axis=0))
                                gts.append(gt)
                            st_ps = pst_pool.tile([128, F], bf16, tag="st")
                            for gg in range(cfg.gpc):
                                for dd in range(D):
                                    nc.tensor.matmul(
                                        st_ps[:, gg * 128:(gg + 1) * 128],
                                        lhsT=gts[gg][:, dd * 128:(dd + 1) * 128],
                                        rhs=identb[:], is_transpose=True,
                                        start=(dd == 0), stop=(dd == D - 1))
                            stbf = isb.tile([128, F], bf16, tag="stbf")
                            nc.vector.tensor_copy(stbf[:], st_ps[:])
                        ne_ps = pne.tile([128, F], f32, tag="ne")
                        nc.tensor.matmul(ne_ps[:], lhsT=w5tb[:], rhs=stbf[:],
                                         start=True, stop=True)
                        nebf = isb.tile([128, F], bf16, tag="nebf")
                        nc.scalar.activation(nebf[:], ne_ps[:], AF.Tanh,
                                             bias=bcol_sb[:, 2:3])
                        iin = isb.tile([128, F], f32, tag="iin")
                        nc.vector.scalar_tensor_tensor(
                            iin[:], in0=nebf[:], scalar=sc["wc2"],
                            in1=PB[:, cs], op0=OP.mult, op1=OP.add)
                        infobf = isb.tile([128, F], bf16, tag="infobf")
                        nc.scalar.activation(infobf[:], iin[:], AF.Tanh)
                        vtbf = isb.tile([128, F], bf16, tag="vtbf")
                        nc.vector.tensor_copy(vtbf[:], VT[:, cs])
                        # gates: keep/upd in one 2-bank psum, new separate
                        ku = pku.tile([128, 2 * F], f32, tag="ku")
                        nc.tensor.matmul(ku[:, :F], lhsT=burow_sb[0][:],
                                         rhs=ones_row[:], start=True, stop=False)
                        nc.tensor.matmul(ku[:, F:], lhsT=burow_sb[1][:],
                                         rhs=ones_row[:], start=True, stop=False)
                        nc.tensor.matmul(ku[:, :F], lhsT=wutb0[:, 0:E],
                                         rhs=vtbf[:], start=False, stop=False)
                        nc.tensor.matmul(ku[:, :F], lhsT=wutb1[:, 0:E],
                                         rhs=infobf[:], start=False, stop=True)
                        nc.tensor.matmul(ku[:, F:], lhsT=wutb0[:, E:2 * E],
                                         rhs=vtbf[:], start=False, stop=False)
                        nc.tensor.matmul(ku[:, F:], lhsT=wutb1[:, E:2 * E],
                                         rhs=infobf[:], start=False, stop=True)
                        nw = pnw.tile([128, F], f32, tag="nw")
                        nc.tensor.matmul(nw[:], lhsT=wutb0[:, 2 * E:3 * E],
                                         rhs=vtbf[:], start=True, stop=False)
                        nc.tensor.matmul(nw[:], lhsT=wutb1[:, 2 * E:3 * E],
                                         rhs=infobf[:], start=False, stop=True)
                        sig = isb.tile([128, 2 * F], f32, tag="sig")
                        nc.scalar.activation(sig[:], ku[:], AF.Sigmoid)
                        tanN = isb.tile([128, F], f32, tag="tanN")
                        nc.scalar.activation(tanN[:], nw[:], AF.Tanh,
                                             bias=bcol_sb[:, 3:4])
                        m1 = isb.tile([128, F], f32, tag="m1")
                        nc.vector.tensor_tensor(m1[:], VT[:, cs], sig[:, :F],
                                                op=OP.mult)
                        m2 = isb.tile([128, F], f32, tag="m2")
                        nc.vector.tensor_tensor(m2[:], sig[:, F:], tanN[:],
                                                op=OP.mult)
                        s_t = isb.tile([128, F], f32, tag="s_t")
                        nc.vector.tensor_tensor(s_t[:], m1[:], m2[:], op=OP.add)
                        nc.scalar.activation(VT[:, cs], s_t[:], AF.Tanh)
                        if t < cfg.t_iters - 1:
                            vnbf = isb.tile([128, F], bf16, tag="vnbf")
                            nc.vector.tensor_copy(vnbf[:], VT[:, cs])
                            vo = pvo.tile([128, F], bf16, tag="vo")
                            for gg in range(cfg.gpc):
                                nc.tensor.matmul(
                                    vo[:, gg * 128:(gg + 1) * 128],
                                    lhsT=vnbf[:, gg * 128:(gg + 1) * 128],
                                    rhs=identb[:], is_transpose=True,
                                    start=True, stop=True)
                            vnode = isb.tile([128, F], bf16, tag="vnode")
                            nc.vector.tensor_copy(vnode[:], vo[:])
                            nc.sync.dma_start(out=Vsh[t][c], in_=vnode[:])
                            if (c + 1) % cfg.cpp == 0:
                                q = c // cfg.cpp
                                rpp = cfg.cpp * cfg.f   # rows per piece/rank
                                nc.gpsimd.collective_compute(
                                    "AllGather", OP.bypass,
                                    replica_groups=rg,
                                    ins=[Vsh[t][q * cfg.cpp:(q + 1) * cfg.cpp]],
                                    outs=[Vfull[t][q * cfg.ncores * rpp:
                                                   (q + 1) * cfg.ncores * rpp, :]])
                        else:
                            vo = pku.tile([128, F], f32, tag="ku", name="vof")
                            for gg in range(cfg.gpc):
                                nc.tensor.matmul(
                                    vo[:, gg * 128:(gg + 1) * 128],
                                    lhsT=VT[:, c * F + gg * 128:
                                            c * F + (gg + 1) * 128],
                                    rhs=identf[:], is_transpose=True,
                                    start=True, stop=True)
                            vout_sb = isb.tile([128, F], f32, tag="vout")
                            nc.vector.tensor_copy(vout_sb[:], vo[:])
                            nc.sync.dma_start(out=Vout[c], in_=vout_sb[:])
                # grid embedding partial
                red = isb.tile([128, 1], f32, tag="red")
                nc.vector.tensor_reduce(red[:], VT[:, 0:cfg.m_real],
                                        axis=mybir.AxisListType.X, op=OP.add)
                g_ps = pne.tile([128, 1], f32, tag="ne", name="g_ps")
                nc.tensor.matmul(g_ps[:], lhsT=wgt[:], rhs=red[:],
                                 start=True, stop=True)
                g_sb2 = isb.tile([128, 1], f32, tag="gsb")
                nc.vector.tensor_copy(g_sb2[:], g_ps[:])
                nc.sync.dma_start(out=gpart[:], in_=g_sb2[:])
    if not nc.is_finalized():
        nc.finalize()
    return nc


# ---------------------------------------------------------------- run

def _assemble(cfg, results, scalars):
    vparts = []
    gsum = np.zeros(cfg.e, np.float64)
    for r in results:
        vt = r["Vout"]  # [nchunk,128,F]
        v = vt.reshape(cfg.nchunk, 128, cfg.gpc, 128).transpose(0, 2, 1, 3)
        v = v.reshape(cfg.mp, cfg.e)[:cfg.m_real]
        vparts.append(v)
        gsum += r["gpart"][:, 0].astype(np.float64)
    V = np.concatenate(vparts, axis=0).astype(F32)
    grid = (gsum + np.asarray(scalars["bg"], np.float64)).astype(F32)
    return V, grid


def run(inputs, cfg=FULL, trace=False, **run_kwargs):
    scalars, rep, per_core = prep_host(cfg, inputs)
    nc = build_program(cfg, scalars)
    in_maps = []
    for cidx in range(cfg.ncores):
        m = dict(rep)
        m.update(per_core[cidx])
        in_maps.append(m)
    res = run_bass_kernel_spmd(nc, in_maps, list(range(cfg.ncores)),
                               trace=trace, **run_kwargs)
    V, grid = _assemble(cfg, res.results, scalars)
    return (V, grid), res


def kernel(**inputs):
    (V, grid), _ = run(inputs)
    return V, grid


# revision 16
# speedup vs baseline: 1.1718x; 1.1718x over previous
"""Trainium2 Bass kernel: DeepIterativeNetwork (GNN message passing).

Strategy (8 NeuronCores, SPMD):
  - Data-parallel over the node axis N: each core owns MP=12800 node slots
    (12500 real + 300 pad).
  - All per-node activations live feature-major in SBUF ([128=E partitions,
    nodes on the free axis]); weights are stationary lhsT tiles.
  - Per message-passing iteration, each core writes its V-shard (bf16,
    transpose-tiled layout) to DRAM and AllGathers it piecewise so every core
    sees the full V for the neighbor gather.  The neighbor gather is an
    indirect DMA of 256B rows; the 4-neighbor sum runs as strided bf16 DVE
    adds and the transpose back to feature-major as PE transpose-matmuls.
  - Small outer-product matmuls (preamble rank-1 terms, bias preloads) use
    explicit tile_position row-groups so 3-4 of them execute concurrently
    on the PE array.
  - Host does index-only preprocessing (neighbor ids, layout remap) and
    parameter staging (transpose / cast / stacking); all floating-point math
    on the N-sized data runs on device.
"""

import os
import sys

for _p in ("/opt/trn_rl_repo", "/root/.axon_site/_ro/trn_rl_repo"):
    if os.path.isdir(_p) and _p not in sys.path:
        sys.path.insert(0, _p)

import numpy as np
import ml_dtypes

import concourse.bass as bass
import concourse.bacc as bacc
import concourse.tile as tile
from concourse import mybir
from concourse.bass_utils import run_bass_kernel_spmd

BF16 = ml_dtypes.bfloat16
F32 = np.float32

AF = mybir.ActivationFunctionType
OP = mybir.AluOpType

# ---------------------------------------------------------------- config

class Cfg:
    def __init__(self, ncores=8, m_real=12500, mp=12800, npiece=5, nb=200000,
                 t_iters=3):
        self.ncores = ncores
        self.m_real = m_real          # real nodes per core
        self.mp = mp                  # padded nodes per core (mult of 512)
        self.e = 128
        self.d = 4
        self.t_iters = t_iters
        self.f = 512                  # chunk size (nodes per chunk)
        self.nchunk = mp // self.f
        self.npiece = npiece          # all-gather pieces per iteration
        assert self.nchunk % npiece == 0
        self.cpp = self.nchunk // npiece   # chunks per piece
        self.gpc = self.f // 128           # groups (of 128 nodes) per chunk
        self.nb = nb

    @property
    def n_total(self):
        return self.ncores * self.m_real


FULL = Cfg()

# ---------------------------------------------------------------- host prep

def prep_host(cfg, inputs):
    """Index preprocessing + parameter staging + sharding. Returns
    (scalars_dict, replicated_map, per_core_maps)."""
    N = cfg.n_total
    dev_ids = np.arange(N, dtype=np.int64)
    dbi = np.asarray(inputs["dev_breaker_ids"])
    bp = np.asarray(inputs["breaker_pairs"])
    ends = bp[dbi]                                    # (N, D, 2)
    nbr = np.where(ends[..., 0] == dev_ids[:, None], ends[..., 1],
                   ends[..., 0]).astype(np.int64)     # (N, D)

    # remap neighbor node ids -> rows of the allgathered Vfull buffer
    grow = np.empty(N, dtype=np.int64)
    nn = np.arange(N)
    rho = nn // cfg.m_real
    l = nn % cfg.m_real
    c = l // cfg.f
    q = c // cfg.cpp
    rr = l % cfg.f
    tiled = c * cfg.f + (rr % 128) * cfg.gpc + (rr // 128)
    rpp = cfg.cpp * cfg.f
    grow[:] = (q * cfg.ncores + rho) * rpp + (tiled - q * rpp)
    nbr_rows = grow[nbr]                              # (N, D) int64

    cbs = np.asarray(inputs["breaker_state"])[dbi].astype(F32)   # (N, D)
    ps = np.asarray(inputs["protector_state"]).astype(F32)       # (N, 3)

    # parameters ------------------------------------------------------
    W0 = np.asarray(inputs["W0"], F32); b0 = np.asarray(inputs["b0"], F32)
    w1 = np.asarray(inputs["w1"], F32); b1 = np.asarray(inputs["b1"], F32)
    w2 = np.asarray(inputs["w2"], F32); b2 = np.asarray(inputs["b2"], F32)
    W3 = np.asarray(inputs["W3"], F32); b3 = np.asarray(inputs["b3"], F32)
    w4 = np.asarray(inputs["w4"], F32); b4 = np.asarray(inputs["b4"], F32)
    W5 = np.asarray(inputs["W5"], F32); b5 = np.asarray(inputs["b5"], F32)
    wc = np.asarray(inputs["wc"], F32); bc = float(np.asarray(inputs["bc"]))
    Wu = np.asarray(inputs["Wu"], F32); bu = np.asarray(inputs["bu"], F32)
    Wg = np.asarray(inputs["Wg"], F32); bg = np.asarray(inputs["bg"], F32)
    v0 = np.asarray(inputs["v0"], F32)

    E = cfg.e
    # outer-product stationary tiles, replicated at PE row-groups 0/32/64/96
    awa = np.zeros((98, E), F32)
    for i in range(4):
        awa[32 * i] = w4
        awa[32 * i + 1] = b4
    awb = np.zeros((104, E), F32)
    for j in range(3):
        awb[32 * j] = w1
        awb[32 * j + 1] = b1
    awb[96::2] = w2          # rows 96,98,100,102
    awb[97] = b2             # rows 99,101,103 stay 0
    b30 = np.zeros((34, E), F32)
    b30[0] = b3
    b30[32] = b0
    bu3 = np.zeros((66, E), F32)
    bu3[0] = bu[0:E]
    bu3[32] = bu[E:2 * E]
    bu3[64] = bu[2 * E:]

    rep = {
        "WgT": np.ascontiguousarray(Wg.T),
        "W5Tb": np.ascontiguousarray(W5.T).astype(BF16),
        "W3Tb": np.ascontiguousarray(W3.T).astype(BF16),
        "W0Tb": np.ascontiguousarray(W0.T).astype(BF16),
        "WuTb": np.ascontiguousarray(Wu.T).reshape(2, E, 3 * E).astype(BF16),
        "awa": awa.astype(BF16),
        "awb": awb.astype(BF16),
        "b30": b30.astype(BF16),
        "bu3": bu3.astype(BF16),
        "b5col": b5.reshape(E, 1).astype(F32),
        "v0col": v0.reshape(E, 1).astype(F32),
    }
    scalars = {"wc0": float(wc[0]), "wc1": float(wc[1]), "wc2": float(wc[2]),
               "bc": bc, "bg": bg}

    per_core = []
    for cidx in range(cfg.ncores):
        s = cidx * cfg.m_real
        epad = cfg.mp - cfg.m_real
        cb = np.pad(cbs[s:s + cfg.m_real], ((0, epad), (0, 0))).T  # [4, MP]
        pst = np.pad(ps[s:s + cfg.m_real], ((0, epad), (0, 0))).T  # [3, MP]
        cbc = cb.reshape(4, cfg.nchunk, cfg.f)
        psc = pst.reshape(3, cfg.nchunk, cfg.f)
        pra = np.zeros((cfg.nchunk, 98, cfg.f), F32)
        for i in range(4):
            pra[:, 32 * i] = cbc[i]
            pra[:, 32 * i + 1] = 1.0
        prb = np.zeros((cfg.nchunk, 104, cfg.f), F32)
        for j in range(3):
            prb[:, 32 * j] = psc[j]
            prb[:, 32 * j + 1] = 1.0
        for i in range(4):
            prb[:, 96 + 2 * i] = cbc[i]
            prb[:, 97 + 2 * i] = 1.0
        # gather indices [128, nchunk*gpc*4]
        nr = np.zeros((cfg.mp, cfg.d), np.int64)
        nr[:cfg.m_real] = nbr_rows[s:s + cfg.m_real]
        gidx = nr.reshape(cfg.nchunk, cfg.gpc, 128, cfg.d) \
                 .transpose(2, 0, 1, 3).reshape(128, -1).astype(np.int32)
        per_core.append({
            "pra": pra.astype(BF16), "prb": prb.astype(BF16), "gidx": gidx,
        })
    return scalars, rep, per_core


# ---------------------------------------------------------------- program

def build_program(cfg, sc):
    """Build the SPMD Bass program. sc: wc0..2, bc baked as immediates."""
    E, F, D = cfg.e, cfg.f, cfg.d
    nc = bacc.Bacc(None, num_devices=cfg.ncores)
    f32, bf16, i32 = mybir.dt.float32, mybir.dt.bfloat16, mybir.dt.int32

    # ---- I/O ----
    dp = nc.declare_dram_parameter
    WgT = dp("WgT", [E, E], f32, isOutput=False)
    W5Tb = dp("W5Tb", [E, E], bf16, isOutput=False)
    W3Tb = dp("W3Tb", [E, E], bf16, isOutput=False)
    W0Tb = dp("W0Tb", [E, E], bf16, isOutput=False)
    WuTb = dp("WuTb", [2, E, 3 * E], bf16, isOutput=False)
    awa_d = dp("awa", [98, E], bf16, isOutput=False)
    awb_d = dp("awb", [104, E], bf16, isOutput=False)
    b30_d = dp("b30", [34, E], bf16, isOutput=False)
    bu3_d = dp("bu3", [66, E], bf16, isOutput=False)
    b5col = dp("b5col", [E, 1], f32, isOutput=False)
    v0col = dp("v0col", [E, 1], f32, isOutput=False)
    pra_d = dp("pra", [cfg.nchunk, 98, F], bf16, isOutput=False)
    prb_d = dp("prb", [cfg.nchunk, 104, F], bf16, isOutput=False)
    gidx_d = dp("gidx", [128, cfg.nchunk * cfg.gpc * D], i32, isOutput=False)
    Vout = dp("Vout", [cfg.nchunk, 128, F], f32, isOutput=True)
    gpart = dp("gpart", [E, 1], f32, isOutput=True)

    # ---- internal DRAM ----
    n_ag = cfg.t_iters - 1
    rows_full = cfg.ncores * cfg.mp
    Vsh = [nc.dram_tensor(f"Vsh{t}", [cfg.nchunk, 128, F], bf16)
           for t in range(n_ag)]
    Vfull = [nc.dram_tensor(f"Vfull{t}", [rows_full, E], bf16,
                            addr_space="Shared") for t in range(n_ag)]
    rg = [list(range(cfg.ncores))]

    with tile.TileContext(nc) as tc:
        with tc.tile_pool(name="resid", bufs=1) as resid:
            # residents ------------------------------------------------
            VT = resid.tile([128, cfg.mp], f32)
            VTBF = resid.tile([128, cfg.mp], bf16)
            PB = resid.tile([128, cfg.mp], f32)
            gidx_sb = resid.tile([128, cfg.nchunk * cfg.gpc * D], i32)
            nc.sync.dma_start(out=gidx_sb[:], in_=gidx_d[:])
            wgt = resid.tile([E, E], f32)
            nc.sync.dma_start(out=wgt[:], in_=WgT[:])
            w5tb = resid.tile([E, E], bf16)
            nc.sync.dma_start(out=w5tb[:], in_=W5Tb[:])
            w3tb = resid.tile([E, E], bf16)
            nc.sync.dma_start(out=w3tb[:], in_=W3Tb[:])
            w0tb = resid.tile([E, E], bf16)
            nc.sync.dma_start(out=w0tb[:], in_=W0Tb[:])
            wutb0 = resid.tile([E, 3 * E], bf16)
            nc.sync.dma_start(out=wutb0[:], in_=WuTb[0])
            wutb1 = resid.tile([E, 3 * E], bf16)
            nc.sync.dma_start(out=wutb1[:], in_=WuTb[1])
            awa_sb = resid.tile([98, E], bf16)
            nc.sync.dma_start(out=awa_sb[:], in_=awa_d[:])
            awb_sb = resid.tile([104, E], bf16)
            nc.sync.dma_start(out=awb_sb[:], in_=awb_d[:])
            b30_sb = resid.tile([34, E], bf16)
            nc.sync.dma_start(out=b30_sb[:], in_=b30_d[:])
            bu3_sb = resid.tile([66, E], bf16)
            nc.sync.dma_start(out=bu3_sb[:], in_=bu3_d[:])
            b5_sb = resid.tile([E, 1], f32)
            nc.sync.dma_start(out=b5_sb[:], in_=b5col[:])
            v0_sb = resid.tile([E, 1], f32)
            nc.sync.dma_start(out=v0_sb[:], in_=v0col[:])
            identb = resid.tile([E, E], bf16)
            identf = resid.tile([E, E], f32)
            from concourse.masks import make_identity
            make_identity(nc, identf[:])
            nc.vector.tensor_copy(identb[:], identf[:])
            onesP = resid.tile([66, F], bf16)
            nc.vector.memset(onesP[:], 1.0)

            # init resident V (f32 + bf16)
            nc.vector.tensor_copy(VT[:, :], v0_sb[:].to_broadcast([128, cfg.mp]))
            nc.vector.tensor_copy(VTBF[:, :],
                                  v0_sb[:].to_broadcast([128, cfg.mp]))

            # ---- preamble: PB = wc0*pe + wc1*be + bc ----
            with tc.tile_pool(name="pre_ps", bufs=1, space="PSUM") as pps, \
                 tc.tile_pool(name="pre_sb", bufs=2) as psb:
                for c in range(cfg.nchunk):
                    cs = slice(c * F, (c + 1) * F)
                    pra_t = psb.tile([98, F], bf16, tag="pra_t")
                    nc.sync.dma_start(out=pra_t[:], in_=pra_d[c])
                    prb_t = psb.tile([104, F], bf16, tag="prb_t")
                    nc.sync.dma_start(out=prb_t[:], in_=prb_d[c])
                    # group A: 4 concurrent K=2 outer products (be_d)
                    pa = pps.tile([128, 4 * F], f32, tag="pa")
                    for i in range(4):
                        nc.tensor.matmul(
                            pa[:, i * F:(i + 1) * F],
                            lhsT=awa_sb[32 * i:32 * i + 2, :],
                            rhs=pra_t[32 * i:32 * i + 2, :],
                            start=True, stop=True, tile_position=(32 * i, 0))
                    HA = psb.tile([128, 4 * F], bf16, tag="HA")
                    nc.scalar.activation(HA[:], pa[:], AF.Tanh)
                    # group B: pe_j (K=2) x3 + tb (K=8), concurrent
                    pb4 = pps.tile([128, 4 * F], f32, tag="pb4")
                    for j in range(3):
                        nc.tensor.matmul(
                            pb4[:, j * F:(j + 1) * F],
                            lhsT=awb_sb[32 * j:32 * j + 2, :],
                            rhs=prb_t[32 * j:32 * j + 2, :],
                            start=True, stop=True, tile_position=(32 * j, 0))
                    nc.tensor.matmul(
                        pb4[:, 3 * F:], lhsT=awb_sb[96:104, :],
                        rhs=prb_t[96:104, :],
                        start=True, stop=True, tile_position=(96, 0))
                    HB = psb.tile([128, 4 * F], bf16, tag="HB")
                    nc.scalar.activation(HB[:], pb4[:], AF.Tanh)
                    # A = sum be_d ; P = sum pe_j + 3*tb  (bf16 adds)
                    a1 = psb.tile([128, F], bf16, tag="a1")
                    nc.vector.tensor_tensor(a1[:], HA[:, 0:F], HA[:, F:2 * F],
                                            op=OP.add)
                    a2 = psb.tile([128, F], bf16, tag="a2")
                    nc.vector.tensor_tensor(a2[:], HA[:, 2 * F:3 * F],
                                            HA[:, 3 * F:], op=OP.add)
                    A = psb.tile([128, F], bf16, tag="A")
                    nc.vector.tensor_tensor(A[:], a1[:], a2[:], op=OP.add)
                    p1 = psb.tile([128, F], bf16, tag="p1")
                    nc.vector.tensor_tensor(p1[:], HB[:, 0:F], HB[:, F:2 * F],
                                            op=OP.add)
                    Pp = psb.tile([128, F], bf16, tag="Pp")
                    nc.vector.tensor_tensor(Pp[:], p1[:], HB[:, 2 * F:3 * F],
                                            op=OP.add)
                    P = psb.tile([128, F], bf16, tag="P")
                    nc.vector.scalar_tensor_tensor(
                        P[:], in0=HB[:, 3 * F:], scalar=3.0, in1=Pp[:],
                        op0=OP.mult, op1=OP.add)
                    # be/pe outer matmuls with bias preloads (concurrent K=1)
                    bpp = pps.tile([128, 2 * F], f32, tag="pa", name="bpp")
                    nc.tensor.matmul(bpp[:, :F], lhsT=b30_sb[0:1, :],
                                     rhs=onesP[0:1, :], start=True, stop=False,
                                     tile_position=(0, 0))
                    nc.tensor.matmul(bpp[:, F:], lhsT=b30_sb[32:33, :],
                                     rhs=onesP[32:33, :], start=True,
                                     stop=False, tile_position=(32, 0))
                    nc.tensor.matmul(bpp[:, :F], lhsT=w3tb[:], rhs=A[:],
                                     start=False, stop=True)
                    nc.tensor.matmul(bpp[:, F:], lhsT=w0tb[:], rhs=P[:],
                                     start=False, stop=True)
                    bepe = psb.tile([128, 2 * F], bf16, tag="bepe")
                    nc.scalar.activation(bepe[:], bpp[:], AF.Tanh)
                    t1 = psb.tile([128, F], f32, tag="t1")
                    nc.vector.tensor_scalar(t1[:], bepe[:, F:],
                                            sc["wc0"], sc["bc"],
                                            OP.mult, OP.add)
                    nc.vector.scalar_tensor_tensor(
                        PB[:, cs], in0=bepe[:, :F], scalar=sc["wc1"],
                        in1=t1[:], op0=OP.mult, op1=OP.add)

            # ---- iterations ----
            with tc.tile_pool(name="ps_st", bufs=2, space="PSUM") as pst_pool, \
                 tc.tile_pool(name="ps_ne", bufs=1, space="PSUM") as pne, \
                 tc.tile_pool(name="ps_ku", bufs=1, space="PSUM") as pku, \
                 tc.tile_pool(name="ps_nw", bufs=1, space="PSUM") as pnw, \
                 tc.tile_pool(name="ps_vo", bufs=2, space="PSUM") as pvo, \
                 tc.tile_pool(name="it_sb", bufs=2) as isb, \
                 tc.tile_pool(name="g_sb", bufs=3) as gsb:
                # iteration-0 constants: ne0b = wc2 * tanh(W5 @ (4 v0) + b5)
                st0c = isb.tile([128, 1], bf16, tag="st0c")
                nc.vector.tensor_scalar(st0c[:], v0_sb[:], float(D), None,
                                        OP.mult)
                ne0_ps = pne.tile([128, 1], f32, tag="ne", name="ne0_ps")
                nc.tensor.matmul(ne0_ps[:], lhsT=w5tb[:], rhs=st0c[:],
                                 start=True, stop=True)
                ne0b = isb.tile([128, 1], f32, tag="ne0b")
                nc.scalar.activation(ne0b[:], ne0_ps[:], AF.Tanh, bias=b5_sb[:])
                nc.vector.tensor_scalar(ne0b[:], ne0b[:], sc["wc2"], None,
                                        OP.mult)
                for t in range(cfg.t_iters):
                    vfull_t = Vfull[t - 1] if t > 0 else None
                    for c in range(cfg.nchunk):
                        cs = slice(c * F, (c + 1) * F)
                        if t > 0:
                            G = gsb.tile([128, cfg.gpc * D * 128], bf16,
                                         tag="g")
                            for gg in range(cfg.gpc):
                                col = (c * cfg.gpc + gg) * D
                                nc.gpsimd.indirect_dma_start(
                                    out=G[:, gg * D * 128:(gg + 1) * D * 128],
                                    out_offset=None,
                                    in_=vfull_t[:, :],
                                    in_offset=bass.IndirectOffsetOnAxis(
                                        ap=gidx_sb[:, col:col + D], axis=0))
                            # neighbor sum: strided bf16 adds over d
                            Gv = G[:].rearrange("p (g d e) -> p g d e",
                                                g=cfg.gpc, d=D)
                            sa = isb.tile([128, F], bf16, tag="sa")
                            sav = sa[:].rearrange("p (g e) -> p g e",
                                                  g=cfg.gpc)
                            nc.vector.tensor_tensor(sav, Gv[:, :, 0, :],
                                                    Gv[:, :, 1, :], op=OP.add)
                            sb2 = isb.tile([128, F], bf16, tag="sb2")
                            sbv = sb2[:].rearrange("p (g e) -> p g e",
                                                   g=cfg.gpc)
                            nc.vector.tensor_tensor(sbv, Gv[:, :, 2, :],
                                                    Gv[:, :, 3, :], op=OP.add)
                            S = isb.tile([128, F], bf16, tag="S")
                            nc.vector.tensor_tensor(S[:], sa[:], sb2[:],
                                                    op=OP.add)
                            st_ps = pst_pool.tile([128, F], bf16, tag="st")
                            for gg in range(cfg.gpc):
                                nc.tensor.matmul(
                                    st_ps[:, gg * 128:(gg + 1) * 128],
                                    lhsT=S[:, gg * 128:(gg + 1) * 128],
                                    rhs=identb[:], is_transpose=True,
                                    start=True, stop=True)
                            stbf = isb.tile([128, F], bf16, tag="stbf")
                            nc.vector.tensor_copy(stbf[:], st_ps[:])
                            ne_ps = pne.tile([128, F], f32, tag="ne")
                            nc.tensor.matmul(ne_ps[:], lhsT=w5tb[:],
                                             rhs=stbf[:], start=True,
                                             stop=True)
                            nebf = isb.tile([128, F], bf16, tag="nebf")
                            nc.scalar.activation(nebf[:], ne_ps[:], AF.Tanh,
                                                 bias=b5_sb[:])
                            iin = isb.tile([128, F], f32, tag="iin")
                            nc.vector.scalar_tensor_tensor(
                                iin[:], in0=nebf[:], scalar=sc["wc2"],
                                in1=PB[:, cs], op0=OP.mult, op1=OP.add)
                            infobf = isb.tile([128, F], bf16, tag="infobf")
                            nc.scalar.activation(infobf[:], iin[:], AF.Tanh)
                        else:
                            infobf = isb.tile([128, F], bf16, tag="infobf")
                            nc.scalar.activation(infobf[:], PB[:, cs], AF.Tanh,
                                                 bias=ne0b[:])
                        # gates: bias preloads (3 concurrent K=1) + 6 matmuls
                        ku = pku.tile([128, 2 * F], f32, tag="ku")
                        nw = pnw.tile([128, F], f32, tag="nw")
                        nc.tensor.matmul(ku[:, :F], lhsT=bu3_sb[0:1, :],
                                         rhs=onesP[0:1, :], start=True,
                                         stop=False, tile_position=(0, 0))
                        nc.tensor.matmul(ku[:, F:], lhsT=bu3_sb[32:33, :],
                                         rhs=onesP[32:33, :], start=True,
                                         stop=False, tile_position=(32, 0))
                        nc.tensor.matmul(nw[:], lhsT=bu3_sb[64:65, :],
                                         rhs=onesP[64:65, :], start=True,
                                         stop=False, tile_position=(64, 0))
                        nc.tensor.matmul(ku[:, :F], lhsT=wutb0[:, 0:E],
                                         rhs=VTBF[:, cs], start=False,
                                         stop=False)
                        nc.tensor.matmul(ku[:, :F], lhsT=wutb1[:, 0:E],
                                         rhs=infobf[:], start=False, stop=True)
                        nc.tensor.matmul(ku[:, F:], lhsT=wutb0[:, E:2 * E],
                                         rhs=VTBF[:, cs], start=False,
                                         stop=False)
                        nc.tensor.matmul(ku[:, F:], lhsT=wutb1[:, E:2 * E],
                                         rhs=infobf[:], start=False, stop=True)
                        nc.tensor.matmul(nw[:], lhsT=wutb0[:, 2 * E:3 * E],
                                         rhs=VTBF[:, cs], start=False,
                                         stop=False)
                        nc.tensor.matmul(nw[:], lhsT=wutb1[:, 2 * E:3 * E],
                                         rhs=infobf[:], start=False, stop=True)
                        sig = isb.tile([128, 2 * F], f32, tag="sig")
                        nc.scalar.activation(sig[:], ku[:], AF.Sigmoid)
                        tanN = isb.tile([128, F], f32, tag="tanN")
                        nc.scalar.activation(tanN[:], nw[:], AF.Tanh)
                        m1 = isb.tile([128, F], f32, tag="m1")
                        nc.vector.tensor_tensor(m1[:], VT[:, cs], sig[:, :F],
                                                op=OP.mult)
                        m2 = isb.tile([128, F], f32, tag="m2")
                        nc.vector.tensor_tensor(m2[:], sig[:, F:], tanN[:],
                                                op=OP.mult)
                        s_t = isb.tile([128, F], f32, tag="s_t")
                        nc.vector.tensor_tensor(s_t[:], m1[:], m2[:],
                                                op=OP.add)
                        nc.scalar.activation(VT[:, cs], s_t[:], AF.Tanh)
                        nc.vector.tensor_copy(VTBF[:, cs], VT[:, cs])
                        if t < cfg.t_iters - 1:
                            vo = pvo.tile([128, F], bf16, tag="vo")
                            for gg in range(cfg.gpc):
                                nc.tensor.matmul(
                                    vo[:, gg * 128:(gg + 1) * 128],
                                    lhsT=VTBF[:, c * F + gg * 128:
                                              c * F + (gg + 1) * 128],
                                    rhs=identb[:], is_transpose=True,
                                    start=True, stop=True)
                            vnode = isb.tile([128, F], bf16, tag="vnode")
                            nc.vector.tensor_copy(vnode[:], vo[:])
                            nc.sync.dma_start(out=Vsh[t][c], in_=vnode[:])
                            if (c + 1) % cfg.cpp == 0:
                                q = c // cfg.cpp
                                rpp = cfg.cpp * cfg.f
                                nc.gpsimd.collective_compute(
                                    "AllGather", OP.bypass,
                                    replica_groups=rg,
                                    ins=[Vsh[t][q * cfg.cpp:
                                                (q + 1) * cfg.cpp]],
                                    outs=[Vfull[t][q * cfg.ncores * rpp:
                                                   (q + 1) * cfg.ncores * rpp,
                                                   :]])
                        else:
                            vof = pku.tile([128, F], f32, tag="ku", name="vof")
                            for gg in range(cfg.gpc):
                                nc.tensor.matmul(
                                    vof[:, gg * 128:(gg + 1) * 128],
                                    lhsT=VT[:, c * F + gg * 128:
                                            c * F + (gg + 1) * 128],
                                    rhs=identf[:], is_transpose=True,
                                    start=True, stop=True)
                            vout_sb = isb.tile([128, F], f32, tag="vout")
                            nc.vector.tensor_copy(vout_sb[:], vof[:])
                            nc.sync.dma_start(out=Vout[c], in_=vout_sb[:])
                # grid embedding partial
                red = isb.tile([128, 1], f32, tag="red")
                nc.vector.tensor_reduce(red[:], VT[:, 0:cfg.m_real],
                                        axis=mybir.AxisListType.X, op=OP.add)
                g_ps = pne.tile([128, 1], f32, tag="ne", name="g_ps")
                nc.tensor.matmul(g_ps[:], lhsT=wgt[:], rhs=red[:],
                                 start=True, stop=True)
                g_sb2 = isb.tile([128, 1], f32, tag="gsb")
                nc.vector.tensor_copy(g_sb2[:], g_ps[:])
                nc.sync.dma_start(out=gpart[:], in_=g_sb2[:])
    if not nc.is_finalized():
        nc.finalize()
    return nc


# ---------------------------------------------------------------- run

def _assemble(cfg, results, scalars):
    vparts = []
    gsum = np.zeros(cfg.e, np.float64)
    for r in results:
        vt = r["Vout"]  # [nchunk,128,F]
        v = vt.reshape(cfg.nchunk, 128, cfg.gpc, 128).transpose(0, 2, 1, 3)
        v = v.reshape(cfg.mp, cfg.e)[:cfg.m_real]
        vparts.append(v)
        gsum += r["gpart"][:, 0].astype(np.float64)
    V = np.concatenate(vparts, axis=0).astype(F32)
    grid = (gsum + np.asarray(scalars["bg"], np.float64)).astype(F32)
    return V, grid


def run(inputs, cfg=FULL, trace=False, **run_kwargs):
    scalars, rep, per_core = prep_host(cfg, inputs)
    nc = build_program(cfg, scalars)
    in_maps = []
    for cidx in range(cfg.ncores):
        m = dict(rep)
        m.update(per_core[cidx])
        in_maps.append(m)
    res = run_bass_kernel_spmd(nc, in_maps, list(range(cfg.ncores)),
                               trace=trace, **run_kwargs)
    V, grid = _assemble(cfg, res.results, scalars)
    return (V, grid), res


def kernel(**inputs):
    (V, grid), _ = run(inputs)
    return V, grid


# revision 22
# speedup vs baseline: 1.5066x; 1.2858x over previous
"""Trainium2 Bass kernel: DeepIterativeNetwork (GNN message passing).

Strategy (8 NeuronCores, SPMD):
  - Data-parallel over the node axis N: each core owns MP=12800 node slots
    (12500 real + 300 pad).
  - All per-node activations live feature-major in SBUF ([128=E partitions,
    nodes on the free axis]); weights are stationary lhsT tiles.
  - Per message-passing iteration, each core writes its V-shard (bf16,
    transpose-tiled layout) to DRAM and AllGathers it piecewise so every core
    sees the full V for the neighbor gather.  The preamble (per-node
    embeddings) is fused into iteration 0 so the first AllGather overlaps it.
  - The neighbor gather is an indirect DMA of 256B rows; the 4-neighbor sum
    runs as strided bf16 DVE adds; transpose back to feature-major via PE
    transpose-matmuls.
  - Rank-1 outer products (preamble) use explicit tile_position row-groups
    so 4 run concurrently on the PE array; gate biases preload the same way.
  - Host does index-only preprocessing (neighbor ids, layout remap) and
    parameter staging (transpose / cast / stacking); all floating-point math
    on the N-sized data runs on device.
"""

import os
import sys

for _p in ("/opt/trn_rl_repo", "/root/.axon_site/_ro/trn_rl_repo"):
    if os.path.isdir(_p) and _p not in sys.path:
        sys.path.insert(0, _p)

import numpy as np
import ml_dtypes

import concourse.bass as bass
import concourse.bacc as bacc
import concourse.tile as tile
from concourse import mybir
from concourse.bass_utils import run_bass_kernel_spmd

BF16 = ml_dtypes.bfloat16
F32 = np.float32

AF = mybir.ActivationFunctionType
OP = mybir.AluOpType

# ---------------------------------------------------------------- config

class Cfg:
    def __init__(self, ncores=8, m_real=12500, mp=12800, nb=200000,
                 t_iters=3, piece_chunks=(6, 6, 6, 6, 1)):
        self.ncores = ncores
        self.m_real = m_real          # real nodes per core
        self.mp = mp                  # padded nodes per core (mult of 512)
        self.e = 128
        self.d = 4
        self.t_iters = t_iters
        self.f = 512                  # chunk size (nodes per chunk)
        self.nchunk = mp // self.f
        assert sum(piece_chunks) == self.nchunk
        self.piece_chunks = list(piece_chunks)
        self.piece_start = np.concatenate(
            [[0], np.cumsum(piece_chunks)])[:-1].astype(int)
        # global row base of each piece in the Vfull buffer
        self.piece_base = np.concatenate(
            [[0], np.cumsum([ncores * pc * self.f
                             for pc in piece_chunks])])[:-1].astype(int)
        self.chunk_piece = np.zeros(self.nchunk, int)
        for q, (s, n) in enumerate(zip(self.piece_start, piece_chunks)):
            self.chunk_piece[s:s + n] = q
        self.gpc = self.f // 128           # groups (of 128 nodes) per chunk
        self.nb = nb

    @property
    def n_total(self):
        return self.ncores * self.m_real


FULL = Cfg()

# ---------------------------------------------------------------- host prep

def prep_host(cfg, inputs):
    """Index preprocessing + parameter staging + sharding. Returns
    (scalars_dict, replicated_map, per_core_maps)."""
    N = cfg.n_total
    dev_ids = np.arange(N, dtype=np.int64)
    dbi = np.asarray(inputs["dev_breaker_ids"])
    bp = np.asarray(inputs["breaker_pairs"])
    ends = bp[dbi]                                    # (N, D, 2)
    nbr = np.where(ends[..., 0] == dev_ids[:, None], ends[..., 1],
                   ends[..., 0]).astype(np.int64)     # (N, D)

    # remap neighbor node ids -> rows of the allgathered Vfull buffer
    nn = np.arange(N)
    rho = nn // cfg.m_real
    l = nn % cfg.m_real
    c = l // cfg.f
    q = cfg.chunk_piece[c]
    rr = l % cfg.f
    tiled = c * cfg.f + (rr % 128) * cfg.gpc + (rr // 128)
    within = tiled - cfg.piece_start[q] * cfg.f
    pcq = np.asarray(cfg.piece_chunks)[q]
    grow = cfg.piece_base[q] + rho * (pcq * cfg.f) + within
    nbr_rows = grow[nbr]                              # (N, D) int64

    cbs = np.asarray(inputs["breaker_state"])[dbi].astype(F32)   # (N, D)
    ps = np.asarray(inputs["protector_state"]).astype(F32)       # (N, 3)

    # parameters ------------------------------------------------------
    W0 = np.asarray(inputs["W0"], F32); b0 = np.asarray(inputs["b0"], F32)
    w1 = np.asarray(inputs["w1"], F32); b1 = np.asarray(inputs["b1"], F32)
    w2 = np.asarray(inputs["w2"], F32); b2 = np.asarray(inputs["b2"], F32)
    W3 = np.asarray(inputs["W3"], F32); b3 = np.asarray(inputs["b3"], F32)
    w4 = np.asarray(inputs["w4"], F32); b4 = np.asarray(inputs["b4"], F32)
    W5 = np.asarray(inputs["W5"], F32); b5 = np.asarray(inputs["b5"], F32)
    wc = np.asarray(inputs["wc"], F32); bc = float(np.asarray(inputs["bc"]))
    Wu = np.asarray(inputs["Wu"], F32); bu = np.asarray(inputs["bu"], F32)
    Wg = np.asarray(inputs["Wg"], F32); bg = np.asarray(inputs["bg"], F32)
    v0 = np.asarray(inputs["v0"], F32)

    E = cfg.e
    # outer-product stationary tiles, replicated at PE row-groups 0/32/64/96
    awa = np.zeros((98, E), F32)
    for i in range(4):
        awa[32 * i] = w4
        awa[32 * i + 1] = b4
    awb = np.zeros((104, E), F32)
    for j in range(3):
        awb[32 * j] = w1
        awb[32 * j + 1] = b1
    awb[96::2] = w2          # rows 96,98,100,102
    awb[97] = b2
    buKU = np.zeros((34, E), F32)
    buKU[0] = bu[0:E]
    buKU[32] = bu[E:2 * E]
    # bias columns: b5, b3, b0, buN
    bcol4 = np.stack([b5, b3, b0, bu[2 * E:]], axis=1)

    rep = {
        "WgT": np.ascontiguousarray(Wg.T),
        "W5Tb": np.ascontiguousarray(W5.T).astype(BF16),
        "W3Tb": np.ascontiguousarray(W3.T).astype(BF16),
        "W0Tb": np.ascontiguousarray(W0.T).astype(BF16),
        "WuTb": np.ascontiguousarray(Wu.T).reshape(2, E, 3 * E).astype(BF16),
        "awa": awa.astype(BF16),
        "awb": awb.astype(BF16),
        "buKU": buKU.astype(BF16),
        "bcol4": bcol4.astype(F32),
        "v0col": v0.reshape(E, 1).astype(F32),
    }
    scalars = {"wc0": float(wc[0]), "wc1": float(wc[1]), "wc2": float(wc[2]),
               "bc": bc, "bg": bg}

    per_core = []
    for cidx in range(cfg.ncores):
        s = cidx * cfg.m_real
        epad = cfg.mp - cfg.m_real
        cb = np.pad(cbs[s:s + cfg.m_real], ((0, epad), (0, 0))).T  # [4, MP]
        pst = np.pad(ps[s:s + cfg.m_real], ((0, epad), (0, 0))).T  # [3, MP]
        cbc = cb.reshape(4, cfg.nchunk, cfg.f)
        psc = pst.reshape(3, cfg.nchunk, cfg.f)
        pra = np.zeros((cfg.nchunk, 98, cfg.f), F32)
        for i in range(4):
            pra[:, 32 * i] = cbc[i]
            pra[:, 32 * i + 1] = 1.0
        prb = np.zeros((cfg.nchunk, 104, cfg.f), F32)
        for j in range(3):
            prb[:, 32 * j] = psc[j]
            prb[:, 32 * j + 1] = 1.0
        for i in range(4):
            prb[:, 96 + 2 * i] = cbc[i]
            prb[:, 97 + 2 * i] = 1.0
        # gather indices [128, nchunk*gpc*4]
        nr = np.zeros((cfg.mp, cfg.d), np.int64)
        nr[:cfg.m_real] = nbr_rows[s:s + cfg.m_real]
        gidx = nr.reshape(cfg.nchunk, cfg.gpc, 128, cfg.d) \
                 .transpose(2, 0, 1, 3).reshape(128, -1).astype(np.int32)
        per_core.append({
            "pra": pra.astype(BF16), "prb": prb.astype(BF16), "gidx": gidx,
        })
    return scalars, rep, per_core


# ---------------------------------------------------------------- program

def build_program(cfg, sc):
    """Build the SPMD Bass program. sc: wc0..2, bc baked as immediates."""
    E, F, D = cfg.e, cfg.f, cfg.d
    nc = bacc.Bacc(None, num_devices=cfg.ncores)
    f32, bf16, i32 = mybir.dt.float32, mybir.dt.bfloat16, mybir.dt.int32

    # ---- I/O ----
    dp = nc.declare_dram_parameter
    WgT = dp("WgT", [E, E], f32, isOutput=False)
    W5Tb = dp("W5Tb", [E, E], bf16, isOutput=False)
    W3Tb = dp("W3Tb", [E, E], bf16, isOutput=False)
    W0Tb = dp("W0Tb", [E, E], bf16, isOutput=False)
    WuTb = dp("WuTb", [2, E, 3 * E], bf16, isOutput=False)
    awa_d = dp("awa", [98, E], bf16, isOutput=False)
    awb_d = dp("awb", [104, E], bf16, isOutput=False)
    buKU_d = dp("buKU", [34, E], bf16, isOutput=False)
    bcol4_d = dp("bcol4", [E, 4], f32, isOutput=False)
    v0col = dp("v0col", [E, 1], f32, isOutput=False)
    pra_d = dp("pra", [cfg.nchunk, 98, F], bf16, isOutput=False)
    prb_d = dp("prb", [cfg.nchunk, 104, F], bf16, isOutput=False)
    gidx_d = dp("gidx", [128, cfg.nchunk * cfg.gpc * D], i32, isOutput=False)
    Vout = dp("Vout", [cfg.nchunk, 128, F], f32, isOutput=True)
    gpart = dp("gpart", [E, 1], f32, isOutput=True)

    # ---- internal DRAM ----
    n_ag = cfg.t_iters - 1
    rows_full = cfg.ncores * cfg.mp
    Vsh = [nc.dram_tensor(f"Vsh{t}", [cfg.nchunk, 128, F], bf16)
           for t in range(n_ag)]
    Vfull = [nc.dram_tensor(f"Vfull{t}", [rows_full, E], bf16,
                            addr_space="Shared") for t in range(n_ag)]
    rg = [list(range(cfg.ncores))]

    def fire_ag(t, c):
        if t < cfg.t_iters - 1 and c in (cfg.piece_start +
                                         np.asarray(cfg.piece_chunks) - 1):
            q = int(cfg.chunk_piece[c])
            s0 = int(cfg.piece_start[q])
            pc = int(cfg.piece_chunks[q])
            base = int(cfg.piece_base[q])
            nc.gpsimd.collective_compute(
                "AllGather", OP.bypass, replica_groups=rg,
                ins=[Vsh[t][s0:s0 + pc]],
                outs=[Vfull[t][base:base + cfg.ncores * pc * cfg.f, :]])

    with tile.TileContext(nc) as tc:
        with tc.tile_pool(name="resid", bufs=1) as resid:
            # residents ------------------------------------------------
            VTBF = resid.tile([128, cfg.mp], bf16)
            PB = resid.tile([128, cfg.mp], f32)
            gidx_sb = resid.tile([128, cfg.nchunk * cfg.gpc * D], i32)
            nc.sync.dma_start(out=gidx_sb[:], in_=gidx_d[:])
            wgt = resid.tile([E, E], f32)
            nc.sync.dma_start(out=wgt[:], in_=WgT[:])
            w5tb = resid.tile([E, E], bf16)
            nc.sync.dma_start(out=w5tb[:], in_=W5Tb[:])
            w3tb = resid.tile([E, E], bf16)
            nc.sync.dma_start(out=w3tb[:], in_=W3Tb[:])
            w0tb = resid.tile([E, E], bf16)
            nc.sync.dma_start(out=w0tb[:], in_=W0Tb[:])
            wutb0 = resid.tile([E, 3 * E], bf16)
            nc.sync.dma_start(out=wutb0[:], in_=WuTb[0])
            wutb1 = resid.tile([E, 3 * E], bf16)
            nc.sync.dma_start(out=wutb1[:], in_=WuTb[1])
            awa_sb = resid.tile([98, E], bf16)
            nc.sync.dma_start(out=awa_sb[:], in_=awa_d[:])
            awb_sb = resid.tile([104, E], bf16)
            nc.sync.dma_start(out=awb_sb[:], in_=awb_d[:])
            buKU_sb = resid.tile([34, E], bf16)
            nc.sync.dma_start(out=buKU_sb[:], in_=buKU_d[:])
            bcol = resid.tile([E, 4], f32)
            nc.sync.dma_start(out=bcol[:], in_=bcol4_d[:])
            v0_sb = resid.tile([E, 1], f32)
            nc.sync.dma_start(out=v0_sb[:], in_=v0col[:])
            identb = resid.tile([E, E], bf16)
            identf = resid.tile([E, E], f32)
            from concourse.masks import make_identity
            make_identity(nc, identf[:])
            nc.vector.tensor_copy(identb[:], identf[:])
            onesKU = resid.tile([34, F], bf16)
            nc.vector.memset(onesKU[:], 1.0)
            gacc = resid.tile([E, 1], f32)
            nc.vector.memset(gacc[:], 0.0)

            nc.vector.tensor_copy(VTBF[:, :],
                                  v0_sb[:].to_broadcast([128, cfg.mp]))

            def gates_and_update(c, infobf, isb, pool_ku, pool_nw):
                """Common tail: gate matmuls + GRU update for chunk c."""
                cs = slice(c * F, (c + 1) * F)
                ku = pool_ku.tile([128, 2 * F], f32, tag="ku", name="ku")
                nw = pool_nw.tile([128, F], f32, tag="nw", name="nw")
                # K/U bias preloads, concurrent K=1 row-groups
                nc.tensor.matmul(ku[:, :F], lhsT=buKU_sb[0:1, :],
                                 rhs=onesKU[0:1, :], start=True, stop=False,
                                 tile_position=(0, 0))
                nc.tensor.matmul(ku[:, F:], lhsT=buKU_sb[32:33, :],
                                 rhs=onesKU[32:33, :], start=True, stop=False,
                                 tile_position=(32, 0))
                nc.tensor.matmul(ku[:, :F], lhsT=wutb0[:, 0:E],
                                 rhs=VTBF[:, cs], start=False, stop=False)
                nc.tensor.matmul(ku[:, :F], lhsT=wutb1[:, 0:E],
                                 rhs=infobf[:], start=False, stop=True)
                nc.tensor.matmul(ku[:, F:], lhsT=wutb0[:, E:2 * E],
                                 rhs=VTBF[:, cs], start=False, stop=False)
                nc.tensor.matmul(ku[:, F:], lhsT=wutb1[:, E:2 * E],
                                 rhs=infobf[:], start=False, stop=True)
                nc.tensor.matmul(nw[:], lhsT=wutb0[:, 2 * E:3 * E],
                                 rhs=VTBF[:, cs], start=True, stop=False)
                nc.tensor.matmul(nw[:], lhsT=wutb1[:, 2 * E:3 * E],
                                 rhs=infobf[:], start=False, stop=True)
                sig = isb.tile([128, 2 * F], f32, tag="sig", name="sig",
                               bufs=2)
                nc.scalar.activation(sig[:], ku[:], AF.Sigmoid)
                tanN = isb.tile([128, F], f32, tag="tanN", name="tanN")
                nc.scalar.activation(tanN[:], nw[:], AF.Tanh,
                                     bias=bcol[:, 3:4])
                m1 = isb.tile([128, F], f32, tag="m1", name="m1")
                nc.vector.tensor_tensor(m1[:], VTBF[:, cs], sig[:, :F],
                                        op=OP.mult)
                m2 = isb.tile([128, F], f32, tag="m2", name="m2")
                nc.vector.tensor_tensor(m2[:], sig[:, F:], tanN[:],
                                        op=OP.mult)
                s_t = isb.tile([128, F], f32, tag="s_t", name="s_t")
                nc.vector.tensor_tensor(s_t[:], m1[:], m2[:], op=OP.add)
                return s_t

            def store_shard(t, c, isb, ptile_pool, tag):
                """Transpose VTBF chunk -> node-major bf16 -> Vsh[t][c]."""
                vo = ptile_pool.tile([128, F], bf16, tag=tag, name="vo")
                for gg in range(cfg.gpc):
                    nc.tensor.matmul(
                        vo[:, gg * 128:(gg + 1) * 128],
                        lhsT=VTBF[:, c * F + gg * 128:c * F + (gg + 1) * 128],
                        rhs=identb[:], is_transpose=True,
                        start=True, stop=True)
                vnode = isb.tile([128, F], bf16, tag="vnode", name="vnode")
                nc.vector.tensor_copy(vnode[:], vo[:])
                nc.sync.dma_start(out=Vsh[t][c], in_=vnode[:])
                fire_ag(t, c)

            # ---- phase 0: preamble fused with iteration 0 ----
            with tc.tile_pool(name="p0_ps", bufs=1, space="PSUM") as pp0, \
                 tc.tile_pool(name="p0_sb", bufs=2) as psb, \
                 tc.tile_pool(name="i0_sb", bufs=2) as isb0:
                # iteration-0 constant: ne0b = wc2 * tanh(W5 @ (4 v0) + b5)
                st0c = isb0.tile([128, 1], bf16, tag="st0c")
                nc.vector.tensor_scalar(st0c[:], v0_sb[:], float(D), None,
                                        OP.mult)
                ne0_ps = pp0.tile([128, 1], f32, tag="bpp", name="ne0_ps")
                nc.tensor.matmul(ne0_ps[:], lhsT=w5tb[:], rhs=st0c[:],
                                 start=True, stop=True)
                ne0b = isb0.tile([128, 1], f32, tag="ne0b")
                nc.scalar.activation(ne0b[:], ne0_ps[:], AF.Tanh,
                                     bias=bcol[:, 0:1])
                nc.vector.tensor_scalar(ne0b[:], ne0b[:], sc["wc2"], None,
                                        OP.mult)
                for c in range(cfg.nchunk):
                    cs = slice(c * F, (c + 1) * F)
                    pra_t = psb.tile([98, F], bf16, tag="pra_t")
                    nc.sync.dma_start(out=pra_t[:], in_=pra_d[c])
                    prb_t = psb.tile([104, F], bf16, tag="prb_t")
                    nc.sync.dma_start(out=prb_t[:], in_=prb_d[c])
                    # group A: 4 concurrent K=2 outer products (be_d)
                    pa = pp0.tile([128, 4 * F], f32, tag="pa8", name="pa")
                    for i in range(4):
                        nc.tensor.matmul(
                            pa[:, i * F:(i + 1) * F],
                            lhsT=awa_sb[32 * i:32 * i + 2, :],
                            rhs=pra_t[32 * i:32 * i + 2, :],
                            start=True, stop=True, tile_position=(32 * i, 0))
                    HA = psb.tile([128, 4 * F], bf16, tag="HA")
                    nc.scalar.activation(HA[:], pa[:], AF.Tanh)
                    # group B: pe_j (K=2) x3 + tb (K=8), concurrent
                    pb4 = pp0.tile([128, 4 * F], f32, tag="pa8", name="pb4")
                    for j in range(3):
                        nc.tensor.matmul(
                            pb4[:, j * F:(j + 1) * F],
                            lhsT=awb_sb[32 * j:32 * j + 2, :],
                            rhs=prb_t[32 * j:32 * j + 2, :],
                            start=True, stop=True, tile_position=(32 * j, 0))
                    nc.tensor.matmul(
                        pb4[:, 3 * F:], lhsT=awb_sb[96:104, :],
                        rhs=prb_t[96:104, :],
                        start=True, stop=True, tile_position=(96, 0))
                    HB = psb.tile([128, 4 * F], bf16, tag="HB")
                    nc.scalar.activation(HB[:], pb4[:], AF.Tanh)
                    # A = sum be_d ; P = sum pe_j + 3*tb  (bf16 adds)
                    a1 = psb.tile([128, F], bf16, tag="a1")
                    nc.vector.tensor_tensor(a1[:], HA[:, 0:F], HA[:, F:2 * F],
                                            op=OP.add)
                    a2 = psb.tile([128, F], bf16, tag="a2")
                    nc.vector.tensor_tensor(a2[:], HA[:, 2 * F:3 * F],
                                            HA[:, 3 * F:], op=OP.add)
                    A = psb.tile([128, F], bf16, tag="A")
                    nc.vector.tensor_tensor(A[:], a1[:], a2[:], op=OP.add)
                    p1 = psb.tile([128, F], bf16, tag="p1")
                    nc.vector.tensor_tensor(p1[:], HB[:, 0:F], HB[:, F:2 * F],
                                            op=OP.add)
                    Pp = psb.tile([128, F], bf16, tag="Pp")
                    nc.vector.tensor_tensor(Pp[:], p1[:], HB[:, 2 * F:3 * F],
                                            op=OP.add)
                    P = psb.tile([128, F], bf16, tag="P")
                    nc.vector.scalar_tensor_tensor(
                        P[:], in0=HB[:, 3 * F:], scalar=3.0, in1=Pp[:],
                        op0=OP.mult, op1=OP.add)
                    # be/pe outer matmuls (sequential 1-bank psum)
                    bepe = psb.tile([128, 2 * F], bf16, tag="bepe")
                    bppA = pp0.tile([128, F], f32, tag="bpp", name="bppA")
                    nc.tensor.matmul(bppA[:], lhsT=w3tb[:], rhs=A[:],
                                     start=True, stop=True)
                    nc.scalar.activation(bepe[:, :F], bppA[:], AF.Tanh,
                                         bias=bcol[:, 1:2])
                    bppB = pp0.tile([128, F], f32, tag="bpp", name="bppB")
                    nc.tensor.matmul(bppB[:], lhsT=w0tb[:], rhs=P[:],
                                     start=True, stop=True)
                    nc.scalar.activation(bepe[:, F:], bppB[:], AF.Tanh,
                                         bias=bcol[:, 2:3])
                    t1 = psb.tile([128, F], f32, tag="t1")
                    nc.vector.tensor_scalar(t1[:], bepe[:, F:],
                                            sc["wc0"], sc["bc"],
                                            OP.mult, OP.add)
                    nc.vector.scalar_tensor_tensor(
                        PB[:, cs], in0=bepe[:, :F], scalar=sc["wc1"],
                        in1=t1[:], op0=OP.mult, op1=OP.add)
                    # ---- iteration 0 part ----
                    infobf = isb0.tile([128, F], bf16, tag="infobf",
                                       name="infobf")
                    nc.scalar.activation(infobf[:], PB[:, cs], AF.Tanh,
                                         bias=ne0b[:])
                    s_t = gates_and_update(c, infobf, isb0, pp0, pp0)
                    nc.scalar.activation(VTBF[:, cs], s_t[:], AF.Tanh)
                    store_shard(0, c, isb0, pp0, "nw")

            # ---- iterations 1..T-1 ----
            with tc.tile_pool(name="ps_st", bufs=2, space="PSUM") as pst_pool, \
                 tc.tile_pool(name="ps_ne", bufs=2, space="PSUM") as pne, \
                 tc.tile_pool(name="ps_ku", bufs=1, space="PSUM") as pku, \
                 tc.tile_pool(name="ps_nw", bufs=2, space="PSUM") as pnw, \
                 tc.tile_pool(name="it_sb", bufs=3) as isb, \
                 tc.tile_pool(name="g_sb", bufs=3) as gsb:
                for t in range(1, cfg.t_iters):
                    last = (t == cfg.t_iters - 1)
                    vfull_t = Vfull[t - 1]
                    for c in range(cfg.nchunk):
                        cs = slice(c * F, (c + 1) * F)
                        G = gsb.tile([128, cfg.gpc * D * 128], bf16, tag="g")
                        for gg in range(cfg.gpc):
                            col = (c * cfg.gpc + gg) * D
                            nc.gpsimd.indirect_dma_start(
                                out=G[:, gg * D * 128:(gg + 1) * D * 128],
                                out_offset=None,
                                in_=vfull_t[:, :],
                                in_offset=bass.IndirectOffsetOnAxis(
                                    ap=gidx_sb[:, col:col + D], axis=0))
                        # neighbor sum: strided bf16 adds over d
                        Gv = G[:].rearrange("p (g d e) -> p g d e",
                                            g=cfg.gpc, d=D)
                        sa = isb.tile([128, F], bf16, tag="sa")
                        sav = sa[:].rearrange("p (g e) -> p g e", g=cfg.gpc)
                        nc.vector.tensor_tensor(sav, Gv[:, :, 0, :],
                                                Gv[:, :, 1, :], op=OP.add)
                        sb2 = isb.tile([128, F], bf16, tag="sb2")
                        sbv = sb2[:].rearrange("p (g e) -> p g e", g=cfg.gpc)
                        nc.vector.tensor_tensor(sbv, Gv[:, :, 2, :],
                                                Gv[:, :, 3, :], op=OP.add)
                        S = isb.tile([128, F], bf16, tag="S")
                        nc.vector.tensor_tensor(S[:], sa[:], sb2[:],
                                                op=OP.add)
                        st_ps = pst_pool.tile([128, F], bf16, tag="st")
                        for gg in range(cfg.gpc):
                            nc.tensor.matmul(
                                st_ps[:, gg * 128:(gg + 1) * 128],
                                lhsT=S[:, gg * 128:(gg + 1) * 128],
                                rhs=identb[:], is_transpose=True,
                                start=True, stop=True)
                        stbf = isb.tile([128, F], bf16, tag="stbf")
                        nc.vector.tensor_copy(stbf[:], st_ps[:])
                        ne_ps = pne.tile([128, F], f32, tag="ne")
                        nc.tensor.matmul(ne_ps[:], lhsT=w5tb[:], rhs=stbf[:],
                                         start=True, stop=True)
                        nebf = isb.tile([128, F], bf16, tag="nebf")
                        nc.scalar.activation(nebf[:], ne_ps[:], AF.Tanh,
                                             bias=bcol[:, 0:1])
                        iin = isb.tile([128, F], f32, tag="iin")
                        nc.vector.scalar_tensor_tensor(
                            iin[:], in0=nebf[:], scalar=sc["wc2"],
                            in1=PB[:, cs], op0=OP.mult, op1=OP.add)
                        infobf = isb.tile([128, F], bf16, tag="infobf")
                        nc.scalar.activation(infobf[:], iin[:], AF.Tanh)
                        s_t = gates_and_update(c, infobf, isb, pku, pnw)
                        if not last:
                            nc.scalar.activation(VTBF[:, cs], s_t[:], AF.Tanh)
                            store_shard(t, c, isb, pnw, "nw")
                        else:
                            vfin = isb.tile([128, F], f32, tag="vfin",
                                            bufs=2)
                            redc = isb.tile([128, 1], f32, tag="redc")
                            nreal = min(F, cfg.m_real - c * F)
                            nc.scalar.activation(vfin[:], s_t[:], AF.Tanh)
                            nc.vector.tensor_reduce(
                                redc[:], vfin[:, :nreal],
                                axis=mybir.AxisListType.X, op=OP.add)
                            nc.vector.tensor_tensor(gacc[:], gacc[:],
                                                    redc[:], op=OP.add)
                            vof = pnw.tile([128, F], f32, tag="nw",
                                           name="vof")
                            for gg in range(cfg.gpc):
                                nc.tensor.matmul(
                                    vof[:, gg * 128:(gg + 1) * 128],
                                    lhsT=vfin[:, gg * 128:(gg + 1) * 128],
                                    rhs=identf[:], is_transpose=True,
                                    start=True, stop=True)
                            vout_sb = isb.tile([128, F], f32, tag="vout",
                                               bufs=2)
                            nc.vector.tensor_copy(vout_sb[:], vof[:])
                            nc.sync.dma_start(out=Vout[c], in_=vout_sb[:])
                # grid embedding partial
                g_ps = pne.tile([128, 1], f32, tag="ne", name="g_ps")
                nc.tensor.matmul(g_ps[:], lhsT=wgt[:], rhs=gacc[:],
                                 start=True, stop=True)
                g_sb2 = isb.tile([128, 1], f32, tag="gsb")
                nc.vector.tensor_copy(g_sb2[:], g_ps[:])
                nc.sync.dma_start(out=gpart[:], in_=g_sb2[:])
    if not nc.is_finalized():
        nc.finalize()
    return nc


# ---------------------------------------------------------------- run

def _assemble(cfg, results, scalars):
    vparts = []
    gsum = np.zeros(cfg.e, np.float64)
    for r in results:
        vt = r["Vout"]  # [nchunk,128,F]
        v = vt.reshape(cfg.nchunk, 128, cfg.gpc, 128).transpose(0, 2, 1, 3)
        v = v.reshape(cfg.mp, cfg.e)[:cfg.m_real]
        vparts.append(v)
        gsum += r["gpart"][:, 0].astype(np.float64)
    V = np.concatenate(vparts, axis=0).astype(F32)
    grid = (gsum + np.asarray(scalars["bg"], np.float64)).astype(F32)
    return V, grid


def run(inputs, cfg=FULL, trace=False, **run_kwargs):
    scalars, rep, per_core = prep_host(cfg, inputs)
    nc = build_program(cfg, scalars)
    in_maps = []
    for cidx in range(cfg.ncores):
        m = dict(rep)
        m.update(per_core[cidx])
        in_maps.append(m)
    res = run_bass_kernel_spmd(nc, in_maps, list(range(cfg.ncores)),
                               trace=trace, **run_kwargs)
    V, grid = _assemble(cfg, res.results, scalars)
    return (V, grid), res


def kernel(**inputs):
    (V, grid), _ = run(inputs)
    return V, grid
